# revision 15
# baseline (speedup 1.0000x reference)
"""Causal varlen self-attention (packed, equal-length) on 8 trn2 NeuronCores.

Sharding: tensor-parallel over heads — 16 heads / 8 cores = 2 heads per core.
Each core computes qkv + RoPE + RMSNorm + causal attention + sigmoid gating for
its 2 heads over all 4096 tokens, plus its partial output projection
(attn_chunk @ Wo_chunk.T) in bf16.  The host sums the 8 partial outputs in f32.

Per-core pipeline (feature-major q/k: head_dim on partitions):
  - qkv: q,k feature-major [d, t]; v (+ 2 gate logits as extra columns of the
    v weight block) token-major [t, d].  Gates go through ACT Sigmoid into a
    small token-major tile; a tiny PE transpose later turns them into
    free-major rows consumed straight from PSUM (no DRAM round-trip).
  - RoPE: the pair-rotation is a partition-half swap.  The q/k feature rows
    are permuted host-side so each rope pair sits inside one 32-partition
    quadrant, which makes the rotation a single DVE stream_shuffle; the sign
    of the second half is folded into the sin rows of the cos/sin constants.
    RMSNorm stats come from PRE-rope values (rotation preserves sum q^2).
  - scores computed TRANSPOSED: scoresT[s, t] = kf.T @ qf so the k-side
    softmax scale folds into the exp's per-partition scale and transposed
    probs feed the PV matmul (lhsT = token-major V) directly.  Softmax
    denominator = accumulated ones-matmuls over the exp tiles.
  - causal mask: diagonal-chunk matmuls are sliced to the unmasked t-range
    and one [128,128] triangle of -1e30 is accumulated on the PE.
  - emission is software-pipelined: wo(s-1) matmuls are interleaved between
    the QK chunks of seq s so the PE has independent work while ACT runs the
    exps; the final seq's wo is emitted per half-seq as soon as the
    normalized attention rows are ready.
"""

import sys

sys.path.insert(0, "/opt/trn_rl_repo")

import numpy as np
import ml_dtypes

import concourse.bass as bass
import concourse.tile as tile
from concourse import bacc, mybir
from concourse.bass_utils import run_bass_kernel_spmd

N_TOK, HID, NH, HD = 4096, 2048, 16, 128
SEQ, NSEQ = 1024, 4
NCORES = 8
EPS = 1e-6
F32, BF16 = mybir.dt.float32, mybir.dt.bfloat16
BF = ml_dtypes.bfloat16
AF = mybir.ActivationFunctionType

# swap the halves of every 32-partition quadrant (DVE stream_shuffle mask)
SHUF_ROT = [(i + 16) % 32 for i in range(32)]
# d-permutation putting rope pair (j, j+64) into one quadrant:
# partition 32q+i   <- x1 dim 16q+i   (i < 16)
# partition 32q+16+i<- x2 dim 16q+i
PERM = np.concatenate(
    [
        np.concatenate([16 * q + np.arange(16), 64 + 16 * q + np.arange(16)])
        for q in range(4)
    ]
).astype(np.int64)

N_WARM = 40  # dummy matmuls ramping the PE p-state before real work lands


def build_nc():
    """One SPMD Bass program; all per-core data arrives via ExternalInputs."""
    nc = bacc.Bacc("TRN2", target_bir_lowering=False, debug=False, num_devices=NCORES)

    xt = nc.dram_tensor("xt", [128, 16, N_TOK], BF16, kind="ExternalInput")
    wqk = nc.dram_tensor("wqk", [128, 4, 16, 128], BF16, kind="ExternalInput")
    wvg = nc.dram_tensor("wvg", [128, 16, 258], BF16, kind="ExternalInput")
    wot = nc.dram_tensor("wot", [128, 2, HID], BF16, kind="ExternalInput")
    cs = nc.dram_tensor("cs", [128, 2, SEQ], BF16, kind="ExternalInput")
    csk = nc.dram_tensor("csk", [128, 2, SEQ], BF16, kind="ExternalInput")
    tri = nc.dram_tensor("tri", [128, 128], BF16, kind="ExternalInput")
    idn = nc.dram_tensor("idn", [128, 128], BF16, kind="ExternalInput")
    gbc = nc.dram_tensor("gbc", [128, 2], F32, kind="ExternalInput")
    out = nc.dram_tensor("out", [N_TOK, HID], BF16, kind="ExternalOutput")

    from contextlib import ExitStack

    with tile.TileContext(nc) as tc:
        with ExitStack() as stack:
            pool = lambda name, bufs, **kw: stack.enter_context(
                tc.tile_pool(name=name, bufs=bufs, **kw)
            )
            consts = pool("consts", 1)
            xtp = pool("xtp", 3)
            qkp = pool("qkp", 2)
            vp = pool("vp", 2)
            finp = pool("finp", 4)
            sqp = pool("sqp", 4)
            rotp = pool("rotp", 2)
            scrp = pool("scrp", 2)
            exp0p = pool("exp0p", 2)
            exp1p = pool("exp1p", 2)
            attnp = pool("attnp", 2)
            obp = pool("obp", 3)
            bcp = pool("bcp", 2)
            rowp = pool("rowp", 3)
            kcp = pool("kcp", 3)
            gsp = pool("gsp", 3)
            gps = pool("gps", 2, space="PSUM")
            bigps = pool("bigps", 2, space="PSUM")
            pvps = pool("pvps", 2, space="PSUM")
            vecps = pool("vecps", 2, space="PSUM")
            # ---- resident constants
            wqk_t = consts.tile([128, 4, 16, 128], BF16)
            wvg_t = consts.tile([128, 16, 258], BF16)
            wot_t = consts.tile([128, 2, HID], BF16)
            cs_t = consts.tile([128, 2, SEQ], BF16)
            csk_t = consts.tile([128, 2, SEQ], BF16)
            tri_t = consts.tile([128, 128], BF16)
            idn_t = consts.tile([128, 128], BF16)
            gbn_t = consts.tile([128, 2], F32)

            ones_bf = consts.tile([128, 1], BF16)
            nc.vector.memset(ones_bf[:], 1.0)
            ones_f = consts.tile([128, 1], F32)
            nc.vector.memset(ones_f[:], 1.0)
            ones_q = consts.tile([128, 1], BF16)  # 2^-7 exact: stats mm -> mean
            nc.vector.memset(ones_q[:], 1.0 / HD)
            eps_t = consts.tile([128, 1], F32)
            nc.vector.memset(eps_t[:], EPS)
            epsh_t = consts.tile([128, 1], F32)
            nc.vector.memset(epsh_t[:], float(HD * EPS))
            wu_t = consts.tile([128, 256], BF16)
            nc.vector.memset(wu_t[:], 0.001)

            # warmup: ramp the PE p-state while the first DMAs land
            for w in range(N_WARM):
                ps = gps.tile([128, 256], F32, tag="proj", name=f"wu{w}")
                nc.tensor.matmul(
                    ps[:], lhsT=wu_t[:, 0:128], rhs=wu_t[:], start=True, stop=True,
                    skip_group_check=True,
                )

            # ---- DMA staging helpers (emission order tunes queue priority)
            def early_consts():
                nc.scalar.dma_start(out=gbn_t[:], in_=gbc[:])
                for m in range(1, 4):
                    nc.scalar.dma_start(out=wqk_t[:, m], in_=wqk[:, m])
                nc.scalar.dma_start(out=wvg_t[:], in_=wvg[:])

            def late_consts():
                nc.scalar.dma_start(out=cs_t[:], in_=cs[:])
                nc.scalar.dma_start(out=csk_t[:], in_=csk[:])
                nc.scalar.dma_start(out=tri_t[:], in_=tri[:])
                nc.scalar.dma_start(out=idn_t[:], in_=idn[:])
                nc.scalar.dma_start(out=wot_t[:], in_=wot[:])

            xtiles = {}

            def load_xtile(nt):
                if nt in xtiles:
                    return xtiles[nt]
                xtile = xtp.tile([128, 16, 512], BF16, tag="xtile", name=f"xt{nt}")
                bounds = (0, 2, 4, 8, 12, 16) if nt == 0 else (0, 4, 8, 12, 16)
                for a, b in zip(bounds, bounds[1:]):
                    nc.sync.dma_start(
                        out=xtile[:, a:b, :],
                        in_=xt[:, a:b, nt * 512 : (nt + 1) * 512],
                    )
                xtiles[nt] = xtile
                return xtile

            # round-robin copy engines for psum->sbuf moves
            # (GPSIMD/Pool cannot access PSUM on this hw)
            _cp = [0]

            def copy_rr(out_ap, in_ap):
                if _cp[0] % 2 == 0:
                    nc.vector.tensor_copy(out=out_ap, in_=in_ap)
                else:
                    nc.scalar.copy(out=out_ap, in_=in_ap)
                _cp[0] += 1

            def qkv_ntile(nt, qk, vt, gsb):
                """project 512 tokens: q,k feature-major; v+gates token-major."""
                half = nt % 2
                xtile = load_xtile(nt)
                for m in range(4):  # q_h0, q_h1, k_h0, k_h1
                    ps = gps.tile([128, 512], F32, tag="proj")
                    for kc in range(16):
                        nc.tensor.matmul(
                            ps[:],
                            lhsT=wqk_t[:, m, kc, :],
                            rhs=xtile[:, kc, :],
                            start=(kc == 0),
                            stop=(kc == 15),
                        )
                    copy_rr(qk[:, m, half * 512 : (half + 1) * 512], ps[:])
                for ti in range(4):  # v + gate logits, token-major, 128 tok each
                    ps = gps.tile([128, 512], F32, tag="proj")
                    for kc in range(16):
                        nc.tensor.matmul(
                            ps[:, 0:258],
                            lhsT=xtile[:, kc, ti * 128 : (ti + 1) * 128],
                            rhs=wvg_t[:, kc, :],
                            start=(kc == 0),
                            stop=(kc == 15),
                        )
                    copy_rr(vt[:, half * 4 + ti, :], ps[:, 0:256])
                    for h in range(2):  # gate as 1+exp(-(z+b)) = 1/sigmoid:
                        # shares the Exp table with attention; the reciprocal
                        # is folded into the denominator reciprocal later
                        nc.scalar.activation(
                            out=gsb[:, ti * 2 + h : ti * 2 + h + 1],
                            in_=ps[:, 256 + h : 257 + h],
                            func=AF.Exp,
                            bias=gbn_t[:, h : h + 1],
                            scale=-1.0,
                        )
                    nc.vector.tensor_scalar_add(
                        out=gsb[:, ti * 2 : ti * 2 + 2],
                        in0=gsb[:, ti * 2 : ti * 2 + 2],
                        scalar1=ones_f[:],
                    )

            def rope_emit(s, qk, sqs, fins):
                """RoPE + stats squares for all 4 head-tensors of seq s.
                Emitted after qkv(2s+1): sq first (unblocks stats matmuls),
                then per-m rotate+combine; k-side combines go to Pool."""
                for m in range(4):
                    nc.vector.tensor_mul(
                        out=sqs[m][:], in0=qk[:, m, :], in1=qk[:, m, :]
                    )
                for m in (0, 2, 1, 3):  # h0 q,k first: QK(h0,tt0) unblocks early
                    is_q = m < 2
                    cst = cs_t if is_q else csk_t
                    eng = nc.vector if is_q else nc.gpsimd
                    rot = rotp.tile([128, SEQ], BF16, tag="rot")
                    nc.vector.stream_shuffle(rot[:], qk[:, m, :], SHUF_ROT)
                    fin = fins[m]
                    tmp = scrp.tile([128, SEQ], BF16, tag="rtmp")
                    eng.tensor_mul(out=fin[:], in0=qk[:, m, :], in1=cst[:, 0, :])
                    eng.tensor_mul(out=tmp[:], in0=rot[:], in1=cst[:, 1, :])
                    eng.tensor_add(out=fin[:], in0=fin[:], in1=tmp[:])

            def stats_emit(s, sqs, fins, kcols):
                """q: sigma_q broadcast-multiplied into fin (bf16 rows).
                k: kcol[128, 8] per head, consumed by the exp scale."""
                for h in range(2):
                    for half in range(2):
                        js = slice(half * 512, half * 512 + 512)
                        qrow = rowp.tile(
                            [1, 512], F32, tag="qrow", name=f"qr{s}_{h}_{half}"
                        )
                        pss = vecps.tile([1, 512], F32, tag="vec")
                        nc.tensor.matmul(
                            pss[:], lhsT=ones_q[:], rhs=sqs[h][:, js],
                            start=True, stop=True,
                        )
                        nc.scalar.activation(
                            out=qrow[:], in_=pss[:], func=AF.Ln,
                            bias=eps_t[0:1, :], scale=1.0,
                        )
                        nc.scalar.activation(
                            out=qrow[:], in_=qrow[:], func=AF.Exp,
                            bias=0.0, scale=-0.5,
                        )
                        bcq = bcp.tile([128, 512], F32, tag="bcq")
                        nc.gpsimd.partition_broadcast(bcq[:], qrow[:])
                        nc.vector.tensor_mul(
                            out=fins[h][:, js], in0=fins[h][:, js], in1=bcq[:]
                        )
                for h in range(2):
                    kcol = kcols[h]
                    for half in range(2):
                        psc = vecps.tile([128, 4], F32, tag="vec")
                        for b in range(4):
                            sc = half * 4 + b
                            nc.tensor.matmul(
                                psc[:, b : b + 1],
                                lhsT=sqs[2 + h][:, sc * 128 : (sc + 1) * 128],
                                rhs=ones_bf[:],
                                start=True,
                                stop=True,
                                skip_group_check=True,
                            )
                        nc.scalar.activation(
                            out=kcol[:, half * 4 : half * 4 + 4], in_=psc[:],
                            func=AF.Ln, bias=epsh_t[:], scale=1.0,
                        )
                        nc.scalar.activation(
                            out=kcol[:, half * 4 : half * 4 + 4],
                            in_=kcol[:, half * 4 : half * 4 + 4],
                            func=AF.Exp, bias=0.0, scale=-0.5,
                        )

            def make_wo_units(s, att, eng_cycle):
                """wo(s) as a list of emission thunks: 8 token-blocks x 4
                feature-tiles, two matmuls each, cast-copy to bf16 staging,
                one coarse DMA per token-block."""
                units = []
                obs = {}

                def unit(t8, ot, eng):
                    def emit():
                        if t8 not in obs:
                            obs[t8] = obp.tile(
                                [128, HID], BF16, tag="ob", name=f"ob{s}_{t8}"
                            )
                        ob = obs[t8]
                        ts_ = slice(t8 * 128, (t8 + 1) * 128)
                        os_ = slice(ot * 512, (ot + 1) * 512)
                        ps = gps.tile([128, 512], F32, tag="proj")
                        for h in range(2):
                            nc.tensor.matmul(
                                ps[:],
                                lhsT=att[:, h, ts_],
                                rhs=wot_t[:, h, os_],
                                start=(h == 0),
                                stop=(h == 1),
                            )
                        if eng is nc.scalar:
                            nc.scalar.copy(out=ob[:, os_], in_=ps[:])
                        else:
                            eng.tensor_copy(out=ob[:, os_], in_=ps[:])
                        if ot == 3:
                            nc.sync.dma_start(
                                out=out[s * SEQ + t8 * 128 : s * SEQ + (t8 + 1) * 128, :],
                                in_=ob[:],
                            )

                    return emit

                i = 0
                for t8 in range(8):
                    for ot in range(4):
                        units.append(unit(t8, ot, eng_cycle[i % len(eng_cycle)]))
                        i += 1
                return units

            def qk_chunk(h, tt, sc, fins, kcols, expt):
                """one transposed-scores chunk + exp"""
                r = sc - 4 * tt
                c0 = 128 * r if r > 0 else 0
                sps = bigps.tile([128, 512], F32, tag="big")
                nc.tensor.matmul(
                    sps[:, 0 : 512 - c0],
                    lhsT=fins[2 + h][:, sc * 128 : (sc + 1) * 128],
                    rhs=fins[h][:, tt * 512 + c0 : (tt + 1) * 512],
                    start=True,
                    stop=(r < 0),
                    skip_group_check=True,
                )
                if r >= 0:  # diagonal chunk: add the -1e30 triangle on the PE
                    nc.tensor.matmul(
                        sps[:, 0:128],
                        lhsT=idn_t[:],
                        rhs=tri_t[:],
                        start=False,
                        stop=True,
                        skip_group_check=True,
                    )
                nc.scalar.activation(
                    out=expt[:, sc, c0:512], in_=sps[:, 0 : 512 - c0],
                    func=AF.Exp, scale=kcols[h][:, sc : sc + 1],
                )

            def pv_group(s, h, tt, vt, att, expt, gsbs):
                """PV + denominator + gate transpose + normalize for (h, tt)."""
                nsc = 4 * (tt + 1)
                pv = pvps.tile([128, 512], F32, tag="pv")
                for sc in range(nsc):
                    r = sc - 4 * tt
                    c0 = 128 * r if r > 0 else 0
                    nc.tensor.matmul(
                        pv[:, c0:512],
                        lhsT=vt[:, sc, h * 128 : (h + 1) * 128],
                        rhs=expt[:, sc, c0:512],
                        start=(sc == 0),
                        stop=(sc == nsc - 1),
                        skip_group_check=True,
                    )
                den = vecps.tile([1, 512], F32, tag="vec")
                for sc in range(nsc):
                    r = sc - 4 * tt
                    c0 = 128 * r if r > 0 else 0
                    nc.tensor.matmul(
                        den[:, c0:512],
                        lhsT=ones_bf[:],
                        rhs=expt[:, sc, c0:512],
                        start=(sc == 0),
                        stop=(sc == nsc - 1),
                        skip_group_check=True,
                    )
                sig = vecps.tile([1, 512], F32, tag="vec")
                gsb = gsbs[tt]
                for ti in range(4):  # gates token-major -> free-major rows
                    nc.tensor.matmul(
                        sig[:, ti * 128 : (ti + 1) * 128],
                        lhsT=gsb[:, ti * 2 + h : ti * 2 + h + 1],
                        rhs=idn_t[:],
                        start=True,
                        stop=True,
                        skip_group_check=True,
                    )
                with tc.high_priority():
                    rden = rowp.tile([1, 512], F32, tag="row", name=f"rd{s}_{h}_{tt}")
                    nc.vector.reciprocal(out=rden[:], in_=den[:])
                    sigr = rowp.tile([1, 512], F32, tag="row", name=f"sg{s}_{h}_{tt}")
                    nc.vector.reciprocal(out=sigr[:], in_=sig[:])
                    drec = rowp.tile([1, 512], F32, tag="row", name=f"dr{s}_{h}_{tt}")
                    nc.vector.tensor_mul(out=drec[:], in0=rden[:], in1=sigr[:])
                    bcg = bcp.tile([128, 512], F32, tag="bcg")
                    nc.gpsimd.partition_broadcast(bcg[:], drec[:])
                    nc.vector.tensor_mul(
                        out=att[:, h, tt * 512 : (tt + 1) * 512], in0=pv[:], in1=bcg[:]
                    )

            # ================= main pipeline =================
            att_prev = None
            for s in range(NSEQ):
                qk = qkp.tile([128, 4, SEQ], BF16, tag="qk", name=f"qk{s}")
                vt = vp.tile([128, 8, 256], BF16, tag="v", name=f"v{s}")
                att = attnp.tile([128, 2, SEQ], BF16, tag="attn")
                gsbs = [
                    gsp.tile([128, 8], BF16, tag="gsb", name=f"g{2 * s + t}")
                    for t in range(2)
                ]
                sqs = [sqp.tile([128, SEQ], BF16, tag="sq", name=f"sq{s}_{i}") for i in range(4)]
                fins = [finp.tile([128, SEQ], BF16, tag="fin", name=f"fin{s}_{i}") for i in range(4)]
                kcols = [kcp.tile([128, 8], F32, tag="kcol", name=f"kc{s}_{i}") for i in range(2)]

                if s == 0:
                    nc.sync.dma_start(out=wqk_t[:, 0], in_=wqk[:, 0])
                    load_xtile(0)
                    early_consts()
                qkv_ntile(2 * s, qk, vt, gsbs[0])
                if s == 0:
                    late_consts()
                qkv_ntile(2 * s + 1, qk, vt, gsbs[1])
                if s + 1 < NSEQ:
                    load_xtile(2 * (s + 1))
                    load_xtile(2 * (s + 1) + 1)

                rope_emit(s, qk, sqs, fins)

                # wo(s-1): first token-blocks as a PE block (covers the DVE
                # rope latency), the rest interleaved between QK chunks
                if att_prev is not None:
                    wo_units = make_wo_units(
                        s - 1, att_prev,
                        (nc.scalar, nc.vector, nc.scalar, nc.vector),
                    )
                else:
                    wo_units = []
                for u in wo_units[0:12]:
                    u()
                wo_rest = wo_units[12:]

                stats_emit(s, sqs, fins, kcols)

                expts = {}
                for h in range(2):
                    expts[(h, 0)] = exp0p.tile(
                        [128, 4, 512], BF16, tag="e0", name=f"e0_{s}_{h}"
                    )
                    expts[(h, 1)] = exp1p.tile(
                        [128, 8, 512], BF16, tag="e1", name=f"e1_{s}_{h}"
                    )

                wi = 0
                chunks = (
                    [(0, 0, sc) for sc in range(4)]
                    + [(1, 0, sc) for sc in range(4)]
                    + [(0, 1, sc) for sc in range(8)]
                    + [(1, 1, sc) for sc in range(8)]
                )
                for h, tt, sc in chunks:
                    qk_chunk(h, tt, sc, fins, kcols, expts[(h, tt)])
                    if wi < len(wo_rest):
                        wo_rest[wi]()
                        wi += 1
                for u in wo_rest[wi:]:
                    u()

                for h, tt in ((0, 0), (1, 0), (0, 1), (1, 1)):
                    pv_group(s, h, tt, vt, att, expts[(h, tt)], gsbs)
                    if s == NSEQ - 1 and (h, tt) == (1, 0):
                        last_units = make_wo_units(
                            s, att, (nc.scalar, nc.vector)
                        )
                        for u in last_units[0:16]:
                            u()
                if s == NSEQ - 1:
                    for u in last_units[16:]:
                        u()
                att_prev = att

    if not nc.is_finalized():
        nc.finalize()
    return nc


_NC_CACHE = None


def _get_nc():
    global _NC_CACHE
    if _NC_CACHE is None:
        _NC_CACHE = build_nc()
    return _NC_CACHE


def prep_inputs(x, Wqkv, Wo, gate_w, gate_b, norm_w, cos_cache, sin_cache,
                cu_seqlens, max_seqlen, position_ids):
    x = np.asarray(x, np.float32)
    Wqkv = np.asarray(Wqkv, np.float32)
    Wo = np.asarray(Wo, np.float32)
    gate_w = np.asarray(gate_w, np.float32)
    gate_b = np.asarray(gate_b, np.float32)
    norm_w = np.asarray(norm_w, np.float32)
    cos_cache = np.asarray(cos_cache, np.float32)
    sin_cache = np.asarray(sin_cache, np.float32)
    pid = np.asarray(position_ids).astype(np.int64)
    cu = np.asarray(cu_seqlens).astype(np.int64)
    assert int(max_seqlen) == SEQ and x.shape == (N_TOK, HID)
    assert np.array_equal(cu, np.arange(NSEQ + 1, dtype=np.int64) * SEQ)
    assert np.array_equal(pid, np.tile(np.arange(SEQ, dtype=np.int64), NSEQ))

    xtf = np.ascontiguousarray(x.T).reshape(16, 128, N_TOK).transpose(1, 0, 2)
    xtf = np.ascontiguousarray(xtf).astype(BF)

    # cos/sin rows in the quadrant-permuted d layout; sin negated on x2 rows
    C = cos_cache[pid[:SEQ]].T  # [64, 1024]
    S = sin_cache[pid[:SEQ]].T
    pair = PERM % 64
    sign = np.where(PERM < 64, 1.0, -1.0).astype(np.float32)
    cs0 = C[pair]
    cs1 = S[pair] * sign[:, None]
    csf = np.stack([cs0, cs1], axis=1).astype(BF)
    w2 = (norm_w * norm_w)[PERM].reshape(128, 1).astype(np.float32)
    cskf = (csf.astype(np.float32) * w2[:, None, :]).astype(BF)

    trif = np.where(
        np.arange(128)[:, None] > np.arange(128)[None, :], np.float32(-1e30), 0.0
    ).astype(BF)
    idnf = np.eye(128, dtype=np.float32).astype(BF)

    in_maps = []
    for c in range(NCORES):
        hs = [2 * c, 2 * c + 1]
        rows = []
        for t in range(2):  # q, k row blocks: apply the rope d-permutation
            for h in hs:
                rows.extend((t * HID + h * HD + PERM).tolist())
        for h in hs:  # v rows unpermuted
            rows.extend(range(2 * HID + h * HD, 2 * HID + (h + 1) * HD))
        wsel = np.concatenate([Wqkv[rows], gate_w[hs]], axis=0)  # [770, 2048]
        wall = np.ascontiguousarray(wsel.T).reshape(16, 128, 770).transpose(1, 0, 2)
        wqkf = np.ascontiguousarray(
            wall[:, :, 0:512].reshape(128, 16, 4, 128).transpose(0, 2, 1, 3)
        ).astype(BF)  # [128, 4(m), 16(kc), 128]
        wvgf = np.ascontiguousarray(wall[:, :, 512:770]).astype(BF)
        wo_sl = np.ascontiguousarray(Wo[:, c * 256 : (c + 1) * 256].T)
        wotf = wo_sl.reshape(2, 128, HID).transpose(1, 0, 2)
        wotf = np.ascontiguousarray(wotf).astype(BF)
        gbf = np.broadcast_to(-gate_b[hs][None, :], (128, 2)).astype(np.float32)
        gbf = np.ascontiguousarray(gbf)
        in_maps.append(
            {"xt": xtf, "wqk": wqkf, "wvg": wvgf, "wot": wotf, "cs": csf,
             "csk": cskf, "tri": trif, "idn": idnf, "gbc": gbf}
        )
    return in_maps


def run(inputs, trace=False):
    in_maps = prep_inputs(**inputs)
    nc = _get_nc()
    res = run_bass_kernel_spmd(nc, in_maps, core_ids=list(range(NCORES)), trace=trace)
    total = np.zeros((N_TOK, HID), np.float32)
    for c in range(NCORES):
        total += res.results[c]["out"].astype(np.float32)
    return total, res


def kernel(**inputs) -> np.ndarray:
    out, _ = run(inputs)
    return out


# revision 16
# speedup vs baseline: 1.1926x; 1.1926x over previous
"""Causal varlen self-attention (packed, equal-length) on 8 trn2 NeuronCores.

Sharding: tensor-parallel over heads — 16 heads / 8 cores = 2 heads per core.
Each core computes qkv + RoPE + RMSNorm + causal attention + sigmoid gating for
its 2 heads over all 4096 tokens, plus its partial output projection
(attn_chunk @ Wo_chunk.T) in bf16.  The host sums the 8 partial outputs in f32.

Per-core pipeline (feature-major q/k: head_dim on partitions):
  - qkv: q,k feature-major [d, t]; v (+ 2 gate logits as extra columns of the
    v weight block) token-major [t, d].  Gates go through ACT Sigmoid into a
    small token-major tile; a tiny PE transpose later turns them into
    free-major rows consumed straight from PSUM (no DRAM round-trip).
  - RoPE: the pair-rotation is a partition-half swap.  The q/k feature rows
    are permuted host-side so each rope pair sits inside one 32-partition
    quadrant, which makes the rotation a single DVE stream_shuffle; the sign
    of the second half is folded into the sin rows of the cos/sin constants.
    RMSNorm stats come from PRE-rope values (rotation preserves sum q^2).
  - scores computed TRANSPOSED: scoresT[s, t] = kf.T @ qf so the k-side
    softmax scale folds into the exp's per-partition scale and transposed
    probs feed the PV matmul (lhsT = token-major V) directly.  Softmax
    denominator = accumulated ones-matmuls over the exp tiles.
  - causal mask: diagonal-chunk matmuls are sliced to the unmasked t-range
    and one [128,128] triangle of -1e30 is accumulated on the PE.
  - emission is software-pipelined: wo(s-1) matmuls are interleaved between
    the QK chunks of seq s so the PE has independent work while ACT runs the
    exps; the final seq's wo is emitted per half-seq as soon as the
    normalized attention rows are ready.
"""

import sys

sys.path.insert(0, "/opt/trn_rl_repo")

import numpy as np
import ml_dtypes

import concourse.bass as bass
import concourse.tile as tile
from concourse import bacc, mybir
from concourse.bass_utils import run_bass_kernel_spmd

N_TOK, HID, NH, HD = 4096, 2048, 16, 128
SEQ, NSEQ = 1024, 4
NCORES = 8
EPS = 1e-6
F32, BF16 = mybir.dt.float32, mybir.dt.bfloat16
BF = ml_dtypes.bfloat16
AF = mybir.ActivationFunctionType

# swap the halves of every 32-partition quadrant (DVE stream_shuffle mask)
SHUF_ROT = [(i + 16) % 32 for i in range(32)]
# d-permutation putting rope pair (j, j+64) into one quadrant:
# partition 32q+i   <- x1 dim 16q+i   (i < 16)
# partition 32q+16+i<- x2 dim 16q+i
PERM = np.concatenate(
    [
        np.concatenate([16 * q + np.arange(16), 64 + 16 * q + np.arange(16)])
        for q in range(4)
    ]
).astype(np.int64)

N_WARM = 40  # dummy matmuls ramping the PE p-state before real work lands


def build_nc():
    """One SPMD Bass program; all per-core data arrives via ExternalInputs."""
    nc = bacc.Bacc("TRN2", target_bir_lowering=False, debug=False, num_devices=NCORES)

    xt = nc.dram_tensor("xt", [128, 16, N_TOK], BF16, kind="ExternalInput")
    wqk = nc.dram_tensor("wqk", [128, 4, 16, 128], BF16, kind="ExternalInput")
    wvg = nc.dram_tensor("wvg", [128, 16, 258], BF16, kind="ExternalInput")
    wot = nc.dram_tensor("wot", [128, 2, HID], BF16, kind="ExternalInput")
    cs = nc.dram_tensor("cs", [128, 2, SEQ], BF16, kind="ExternalInput")
    csk = nc.dram_tensor("csk", [128, 2, SEQ], BF16, kind="ExternalInput")
    tri = nc.dram_tensor("tri", [128, 128], BF16, kind="ExternalInput")
    idn = nc.dram_tensor("idn", [128, 128], BF16, kind="ExternalInput")
    gbc = nc.dram_tensor("gbc", [128, 2], F32, kind="ExternalInput")
    out = nc.dram_tensor("out", [N_TOK, HID], BF16, kind="ExternalOutput")

    from contextlib import ExitStack

    with tile.TileContext(nc) as tc:
        with ExitStack() as stack:
            pool = lambda name, bufs, **kw: stack.enter_context(
                tc.tile_pool(name=name, bufs=bufs, **kw)
            )
            consts = pool("consts", 1)
            xtp = pool("xtp", 3)
            qkp = pool("qkp", 2)
            vp = pool("vp", 2)
            finp = pool("finp", 4)
            sqp = pool("sqp", 4)
            rotp = pool("rotp", 2)
            scrp = pool("scrp", 2)
            exp0p = pool("exp0p", 2)
            exp1p = pool("exp1p", 2)
            attnp = pool("attnp", 2)
            obp = pool("obp", 3)
            bcp = pool("bcp", 2)
            rowp = pool("rowp", 3)
            kcp = pool("kcp", 3)
            gsp = pool("gsp", 3)
            gps = pool("gps", 2, space="PSUM")
            bigps = pool("bigps", 2, space="PSUM")
            pvps = pool("pvps", 2, space="PSUM")
            vecps = pool("vecps", 2, space="PSUM")
            # ---- resident constants
            wqk_t = consts.tile([128, 4, 16, 128], BF16)
            wvg_t = consts.tile([128, 16, 258], BF16)
            wot_t = consts.tile([128, 2, HID], BF16)
            cs_t = consts.tile([128, 2, SEQ], BF16)
            csk_t = consts.tile([128, 2, SEQ], BF16)
            tri_t = consts.tile([128, 128], BF16)
            idn_t = consts.tile([128, 128], BF16)
            gbn_t = consts.tile([128, 2], F32)

            ones_bf = consts.tile([128, 1], BF16)
            nc.vector.memset(ones_bf[:], 1.0)
            ones_f = consts.tile([128, 1], F32)
            nc.vector.memset(ones_f[:], 1.0)
            ones_q = consts.tile([128, 1], BF16)  # 2^-7 exact: stats mm -> mean
            nc.vector.memset(ones_q[:], 1.0 / HD)
            eps_t = consts.tile([128, 1], F32)
            nc.vector.memset(eps_t[:], EPS)
            epsh_t = consts.tile([128, 1], F32)
            nc.vector.memset(epsh_t[:], float(HD * EPS))
            wu_t = consts.tile([128, 256], BF16)
            nc.vector.memset(wu_t[:], 0.001)

            # warmup: ramp the PE p-state while the first DMAs land
            for w in range(N_WARM):
                ps = gps.tile([128, 256], F32, tag="proj", name=f"wu{w}")
                nc.tensor.matmul(
                    ps[:], lhsT=wu_t[:, 0:128], rhs=wu_t[:], start=True, stop=True,
                    skip_group_check=True,
                )

            # ---- DMA staging helpers (emission order tunes queue priority)
            def early_consts():
                nc.scalar.dma_start(out=gbn_t[:], in_=gbc[:])
                for m in range(1, 4):
                    nc.scalar.dma_start(out=wqk_t[:, m], in_=wqk[:, m])
                nc.scalar.dma_start(out=wvg_t[:], in_=wvg[:])

            def late_consts():
                nc.scalar.dma_start(out=cs_t[:], in_=cs[:])
                nc.scalar.dma_start(out=csk_t[:], in_=csk[:])
                nc.scalar.dma_start(out=tri_t[:], in_=tri[:])
                nc.scalar.dma_start(out=idn_t[:], in_=idn[:])
                nc.scalar.dma_start(out=wot_t[:], in_=wot[:])

            xtiles = {}

            def load_xtile(nt):
                if nt in xtiles:
                    return xtiles[nt]
                xtile = xtp.tile([128, 16, 512], BF16, tag="xtile", name=f"xt{nt}")
                bounds = (0, 2, 4, 8, 12, 16) if nt == 0 else (0, 4, 8, 12, 16)
                for a, b in zip(bounds, bounds[1:]):
                    nc.sync.dma_start(
                        out=xtile[:, a:b, :],
                        in_=xt[:, a:b, nt * 512 : (nt + 1) * 512],
                    )
                xtiles[nt] = xtile
                return xtile

            # round-robin copy engines for psum->sbuf moves
            # (GPSIMD/Pool cannot access PSUM on this hw)
            _cp = [0]

            def copy_rr(out_ap, in_ap):
                if _cp[0] % 2 == 0:
                    nc.vector.tensor_copy(out=out_ap, in_=in_ap)
                else:
                    nc.scalar.copy(out=out_ap, in_=in_ap)
                _cp[0] += 1

            def qkv_ntile(nt, qk, vt, gsb):
                """project 512 tokens: q,k feature-major; v+gates token-major."""
                half = nt % 2
                xtile = load_xtile(nt)
                for m in range(4):  # q_h0, q_h1, k_h0, k_h1
                    ps = gps.tile([128, 512], F32, tag="proj")
                    for kc in range(16):
                        nc.tensor.matmul(
                            ps[:],
                            lhsT=wqk_t[:, m, kc, :],
                            rhs=xtile[:, kc, :],
                            start=(kc == 0),
                            stop=(kc == 15),
                        )
                    copy_rr(qk[:, m, half * 512 : (half + 1) * 512], ps[:])
                for ti in range(4):  # v + gate logits, token-major, 128 tok each
                    ps = gps.tile([128, 512], F32, tag="proj")
                    for kc in range(16):
                        nc.tensor.matmul(
                            ps[:, 0:258],
                            lhsT=xtile[:, kc, ti * 128 : (ti + 1) * 128],
                            rhs=wvg_t[:, kc, :],
                            start=(kc == 0),
                            stop=(kc == 15),
                        )
                    copy_rr(vt[:, half * 4 + ti, :], ps[:, 0:256])
                    for h in range(2):  # gate as 1+exp(-(z+b)) = 1/sigmoid:
                        # shares the Exp table with attention; the reciprocal
                        # is folded into the denominator reciprocal later
                        nc.scalar.activation(
                            out=gsb[:, ti * 2 + h : ti * 2 + h + 1],
                            in_=ps[:, 256 + h : 257 + h],
                            func=AF.Exp,
                            bias=gbn_t[:, h : h + 1],
                            scale=-1.0,
                        )
                    nc.vector.tensor_scalar_add(
                        out=gsb[:, ti * 2 : ti * 2 + 2],
                        in0=gsb[:, ti * 2 : ti * 2 + 2],
                        scalar1=ones_f[:],
                    )

            def rope_emit(s, qk, sqs, fins):
                """RoPE + stats squares for all 4 head-tensors of seq s.
                Emitted after qkv(2s+1): sq first (unblocks stats matmuls),
                then per-m rotate+combine; k-side combines go to Pool."""
                for m in range(4):
                    nc.vector.tensor_mul(
                        out=sqs[m][:], in0=qk[:, m, :], in1=qk[:, m, :]
                    )
                for m in (0, 2, 1, 3):  # h0 q,k first: QK(h0,tt0) unblocks early
                    is_q = m < 2
                    cst = cs_t if is_q else csk_t
                    eng = nc.vector if is_q else nc.gpsimd
                    rot = rotp.tile([128, SEQ], BF16, tag="rot")
                    nc.vector.stream_shuffle(rot[:], qk[:, m, :], SHUF_ROT)
                    fin = fins[m]
                    tmp = scrp.tile([128, SEQ], BF16, tag="rtmp")
                    eng.tensor_mul(out=fin[:], in0=qk[:, m, :], in1=cst[:, 0, :])
                    eng.tensor_mul(out=tmp[:], in0=rot[:], in1=cst[:, 1, :])
                    eng.tensor_add(out=fin[:], in0=fin[:], in1=tmp[:])

            def stats_emit(s, sqs, fins, kcols):
                """q: sigma_q broadcast-multiplied into fin (bf16 rows).
                k: kcol[128, 8] per head, consumed by the exp scale."""
                for h in range(2):
                    for half in range(2):
                        js = slice(half * 512, half * 512 + 512)
                        qrow = rowp.tile(
                            [1, 512], F32, tag="qrow", name=f"qr{s}_{h}_{half}"
                        )
                        pss = vecps.tile([1, 512], F32, tag="vec")
                        nc.tensor.matmul(
                            pss[:], lhsT=ones_q[:], rhs=sqs[h][:, js],
                            start=True, stop=True,
                        )
                        nc.scalar.activation(
                            out=qrow[:], in_=pss[:], func=AF.Sqrt,
                            bias=eps_t[0:1, :], scale=1.0,
                        )
                        nc.vector.reciprocal(out=qrow[:], in_=qrow[:])
                        bcq = bcp.tile([128, 512], F32, tag="bcq")
                        nc.gpsimd.partition_broadcast(bcq[:], qrow[:])
                        nc.vector.tensor_mul(
                            out=fins[h][:, js], in0=fins[h][:, js], in1=bcq[:]
                        )
                for h in range(2):
                    kcol = kcols[h]
                    for half in range(2):
                        psc = vecps.tile([128, 4], F32, tag="vec")
                        for b in range(4):
                            sc = half * 4 + b
                            nc.tensor.matmul(
                                psc[:, b : b + 1],
                                lhsT=sqs[2 + h][:, sc * 128 : (sc + 1) * 128],
                                rhs=ones_bf[:],
                                start=True,
                                stop=True,
                                skip_group_check=True,
                            )
                        nc.scalar.activation(
                            out=kcol[:, half * 4 : half * 4 + 4], in_=psc[:],
                            func=AF.Sqrt, bias=epsh_t[:], scale=1.0,
                        )
                        nc.vector.reciprocal(
                            out=kcol[:, half * 4 : half * 4 + 4],
                            in_=kcol[:, half * 4 : half * 4 + 4],
                        )

            def make_wo_units(s, att, eng_cycle):
                """wo(s) as a list of emission thunks: 8 token-blocks x 4
                feature-tiles, two matmuls each, cast-copy to bf16 staging,
                one coarse DMA per token-block."""
                units = []
                obs = {}

                def unit(t8, ot, eng):
                    def emit():
                        if t8 not in obs:
                            obs[t8] = obp.tile(
                                [128, HID], BF16, tag="ob", name=f"ob{s}_{t8}"
                            )
                        ob = obs[t8]
                        ts_ = slice(t8 * 128, (t8 + 1) * 128)
                        os_ = slice(ot * 512, (ot + 1) * 512)
                        ps = gps.tile([128, 512], F32, tag="proj")
                        for h in range(2):
                            nc.tensor.matmul(
                                ps[:],
                                lhsT=att[:, h, ts_],
                                rhs=wot_t[:, h, os_],
                                start=(h == 0),
                                stop=(h == 1),
                            )
                        if eng is nc.scalar:
                            nc.scalar.copy(out=ob[:, os_], in_=ps[:])
                        else:
                            eng.tensor_copy(out=ob[:, os_], in_=ps[:])
                        if ot == 3:
                            nc.sync.dma_start(
                                out=out[s * SEQ + t8 * 128 : s * SEQ + (t8 + 1) * 128, :],
                                in_=ob[:],
                            )

                    return emit

                i = 0
                for t8 in range(8):
                    for ot in range(4):
                        units.append(unit(t8, ot, eng_cycle[i % len(eng_cycle)]))
                        i += 1
                return units

            def qk_chunk(h, tt, sc, fins, kcols, expt):
                """one transposed-scores chunk + exp"""
                r = sc - 4 * tt
                c0 = 128 * r if r > 0 else 0
                sps = bigps.tile([128, 512], F32, tag="big")
                nc.tensor.matmul(
                    sps[:, 0 : 512 - c0],
                    lhsT=fins[2 + h][:, sc * 128 : (sc + 1) * 128],
                    rhs=fins[h][:, tt * 512 + c0 : (tt + 1) * 512],
                    start=True,
                    stop=(r < 0),
                    skip_group_check=True,
                )
                if r >= 0:  # diagonal chunk: add the -1e30 triangle on the PE
                    nc.tensor.matmul(
                        sps[:, 0:128],
                        lhsT=idn_t[:],
                        rhs=tri_t[:],
                        start=False,
                        stop=True,
                        skip_group_check=True,
                    )
                nc.scalar.activation(
                    out=expt[:, sc, c0:512], in_=sps[:, 0 : 512 - c0],
                    func=AF.Exp, scale=kcols[h][:, sc : sc + 1],
                )

            def pv_group(s, h, tt, vt, att, expt, gsbs):
                """PV + denominator + gate transpose + normalize for (h, tt)."""
                nsc = 4 * (tt + 1)
                pv = pvps.tile([128, 512], F32, tag="pv")
                for sc in range(nsc):
                    r = sc - 4 * tt
                    c0 = 128 * r if r > 0 else 0
                    nc.tensor.matmul(
                        pv[:, c0:512],
                        lhsT=vt[:, sc, h * 128 : (h + 1) * 128],
                        rhs=expt[:, sc, c0:512],
                        start=(sc == 0),
                        stop=(sc == nsc - 1),
                        skip_group_check=True,
                    )
                den = vecps.tile([1, 512], F32, tag="vec")
                for sc in range(nsc):
                    r = sc - 4 * tt
                    c0 = 128 * r if r > 0 else 0
                    nc.tensor.matmul(
                        den[:, c0:512],
                        lhsT=ones_bf[:],
                        rhs=expt[:, sc, c0:512],
                        start=(sc == 0),
                        stop=(sc == nsc - 1),
                        skip_group_check=True,
                    )
                sig = vecps.tile([1, 512], F32, tag="vec")
                gsb = gsbs[tt]
                for ti in range(4):  # gates token-major -> free-major rows
                    nc.tensor.matmul(
                        sig[:, ti * 128 : (ti + 1) * 128],
                        lhsT=gsb[:, ti * 2 + h : ti * 2 + h + 1],
                        rhs=idn_t[:],
                        start=True,
                        stop=True,
                        skip_group_check=True,
                    )
                with tc.high_priority():
                    rden = rowp.tile([1, 512], F32, tag="row", name=f"rd{s}_{h}_{tt}")
                    nc.vector.reciprocal(out=rden[:], in_=den[:])
                    sigr = rowp.tile([1, 512], F32, tag="row", name=f"sg{s}_{h}_{tt}")
                    nc.vector.reciprocal(out=sigr[:], in_=sig[:])
                    drec = rowp.tile([1, 512], F32, tag="row", name=f"dr{s}_{h}_{tt}")
                    nc.vector.tensor_mul(out=drec[:], in0=rden[:], in1=sigr[:])
                    bcg = bcp.tile([128, 512], F32, tag="bcg")
                    nc.gpsimd.partition_broadcast(bcg[:], drec[:])
                    nc.vector.tensor_mul(
                        out=att[:, h, tt * 512 : (tt + 1) * 512], in0=pv[:], in1=bcg[:]
                    )

            # ================= main pipeline =================
            att_prev = None
            for s in range(NSEQ):
                qk = qkp.tile([128, 4, SEQ], BF16, tag="qk", name=f"qk{s}")
                vt = vp.tile([128, 8, 256], BF16, tag="v", name=f"v{s}")
                att = attnp.tile([128, 2, SEQ], BF16, tag="attn")
                gsbs = [
                    gsp.tile([128, 8], BF16, tag="gsb", name=f"g{2 * s + t}")
                    for t in range(2)
                ]
                sqs = [sqp.tile([128, SEQ], BF16, tag="sq", name=f"sq{s}_{i}") for i in range(4)]
                fins = [finp.tile([128, SEQ], BF16, tag="fin", name=f"fin{s}_{i}") for i in range(4)]
                kcols = [kcp.tile([128, 8], F32, tag="kcol", name=f"kc{s}_{i}") for i in range(2)]

                if s == 0:
                    nc.sync.dma_start(out=wqk_t[:, 0], in_=wqk[:, 0])
                    load_xtile(0)
                    early_consts()
                qkv_ntile(2 * s, qk, vt, gsbs[0])
                if s == 0:
                    late_consts()
                qkv_ntile(2 * s + 1, qk, vt, gsbs[1])
                if s + 1 < NSEQ:
                    load_xtile(2 * (s + 1))
                    load_xtile(2 * (s + 1) + 1)

                rope_emit(s, qk, sqs, fins)

                # wo(s-1): first token-blocks as a PE block (covers the DVE
                # rope latency), the rest interleaved between QK chunks
                if att_prev is not None:
                    wo_units = make_wo_units(
                        s - 1, att_prev,
                        (nc.scalar, nc.vector, nc.scalar, nc.vector),
                    )
                else:
                    wo_units = []
                for u in wo_units[0:12]:
                    u()
                wo_rest = wo_units[12:]

                stats_emit(s, sqs, fins, kcols)

                expts = {}
                for h in range(2):
                    expts[(h, 0)] = exp0p.tile(
                        [128, 4, 512], BF16, tag="e0", name=f"e0_{s}_{h}"
                    )
                    expts[(h, 1)] = exp1p.tile(
                        [128, 8, 512], BF16, tag="e1", name=f"e1_{s}_{h}"
                    )

                wi = 0
                chunks = (
                    [(0, 0, sc) for sc in range(4)]
                    + [(1, 0, sc) for sc in range(4)]
                    + [(0, 1, sc) for sc in range(8)]
                    + [(1, 1, sc) for sc in range(8)]
                )
                for h, tt, sc in chunks:
                    qk_chunk(h, tt, sc, fins, kcols, expts[(h, tt)])
                    if wi < len(wo_rest):
                        wo_rest[wi]()
                        wi += 1
                for u in wo_rest[wi:]:
                    u()

                for h, tt in ((0, 0), (1, 0), (0, 1), (1, 1)):
                    pv_group(s, h, tt, vt, att, expts[(h, tt)], gsbs)
                    if s == NSEQ - 1 and (h, tt) == (1, 0):
                        last_units = make_wo_units(
                            s, att, (nc.scalar, nc.vector)
                        )
                        for u in last_units[0:16]:
                            u()
                if s == NSEQ - 1:
                    for u in last_units[16:]:
                        u()
                att_prev = att

    if not nc.is_finalized():
        nc.finalize()
    return nc


_NC_CACHE = None


def _get_nc():
    global _NC_CACHE
    if _NC_CACHE is None:
        _NC_CACHE = build_nc()
    return _NC_CACHE


def prep_inputs(x, Wqkv, Wo, gate_w, gate_b, norm_w, cos_cache, sin_cache,
                cu_seqlens, max_seqlen, position_ids):
    x = np.asarray(x, np.float32)
    Wqkv = np.asarray(Wqkv, np.float32)
    Wo = np.asarray(Wo, np.float32)
    gate_w = np.asarray(gate_w, np.float32)
    gate_b = np.asarray(gate_b, np.float32)
    norm_w = np.asarray(norm_w, np.float32)
    cos_cache = np.asarray(cos_cache, np.float32)
    sin_cache = np.asarray(sin_cache, np.float32)
    pid = np.asarray(position_ids).astype(np.int64)
    cu = np.asarray(cu_seqlens).astype(np.int64)
    assert int(max_seqlen) == SEQ and x.shape == (N_TOK, HID)
    assert np.array_equal(cu, np.arange(NSEQ + 1, dtype=np.int64) * SEQ)
    assert np.array_equal(pid, np.tile(np.arange(SEQ, dtype=np.int64), NSEQ))

    xtf = np.ascontiguousarray(x.T).reshape(16, 128, N_TOK).transpose(1, 0, 2)
    xtf = np.ascontiguousarray(xtf).astype(BF)

    # cos/sin rows in the quadrant-permuted d layout; sin negated on x2 rows
    C = cos_cache[pid[:SEQ]].T  # [64, 1024]
    S = sin_cache[pid[:SEQ]].T
    pair = PERM % 64
    sign = np.where(PERM < 64, 1.0, -1.0).astype(np.float32)
    cs0 = C[pair]
    cs1 = S[pair] * sign[:, None]
    csf = np.stack([cs0, cs1], axis=1).astype(BF)
    w2 = (norm_w * norm_w)[PERM].reshape(128, 1).astype(np.float32)
    cskf = (csf.astype(np.float32) * w2[:, None, :]).astype(BF)

    trif = np.where(
        np.arange(128)[:, None] > np.arange(128)[None, :], np.float32(-1e30), 0.0
    ).astype(BF)
    idnf = np.eye(128, dtype=np.float32).astype(BF)

    in_maps = []
    for c in range(NCORES):
        hs = [2 * c, 2 * c + 1]
        rows = []
        for t in range(2):  # q, k row blocks: apply the rope d-permutation
            for h in hs:
                rows.extend((t * HID + h * HD + PERM).tolist())
        for h in hs:  # v rows unpermuted
            rows.extend(range(2 * HID + h * HD, 2 * HID + (h + 1) * HD))
        wsel = np.concatenate([Wqkv[rows], gate_w[hs]], axis=0)  # [770, 2048]
        wall = np.ascontiguousarray(wsel.T).reshape(16, 128, 770).transpose(1, 0, 2)
        wqkf = np.ascontiguousarray(
            wall[:, :, 0:512].reshape(128, 16, 4, 128).transpose(0, 2, 1, 3)
        ).astype(BF)  # [128, 4(m), 16(kc), 128]
        wvgf = np.ascontiguousarray(wall[:, :, 512:770]).astype(BF)
        wo_sl = np.ascontiguousarray(Wo[:, c * 256 : (c + 1) * 256].T)
        wotf = wo_sl.reshape(2, 128, HID).transpose(1, 0, 2)
        wotf = np.ascontiguousarray(wotf).astype(BF)
        gbf = np.broadcast_to(-gate_b[hs][None, :], (128, 2)).astype(np.float32)
        gbf = np.ascontiguousarray(gbf)
        in_maps.append(
            {"xt": xtf, "wqk": wqkf, "wvg": wvgf, "wot": wotf, "cs": csf,
             "csk": cskf, "tri": trif, "idn": idnf, "gbc": gbf}
        )
    return in_maps


def run(inputs, trace=False):
    in_maps = prep_inputs(**inputs)
    nc = _get_nc()
    res = run_bass_kernel_spmd(nc, in_maps, core_ids=list(range(NCORES)), trace=trace)
    total = np.zeros((N_TOK, HID), np.float32)
    for c in range(NCORES):
        total += res.results[c]["out"].astype(np.float32)
    return total, res


def kernel(**inputs) -> np.ndarray:
    out, _ = run(inputs)
    return out


# revision 17
# speedup vs baseline: 1.2042x; 1.0098x over previous
"""Causal varlen self-attention (packed, equal-length) on 8 trn2 NeuronCores.

Sharding: tensor-parallel over heads — 16 heads / 8 cores = 2 heads per core.
Each core computes qkv + RoPE + RMSNorm + causal attention + sigmoid gating for
its 2 heads over all 4096 tokens, plus its partial output projection
(attn_chunk @ Wo_chunk.T) in bf16.  The host sums the 8 partial outputs in f32.

Per-core pipeline (feature-major q/k: head_dim on partitions):
  - qkv: q,k feature-major [d, t]; v (+ 2 gate logits as extra columns of the
    v weight block) token-major [t, d].  Gates go through ACT Sigmoid into a
    small token-major tile; a tiny PE transpose later turns them into
    free-major rows consumed straight from PSUM (no DRAM round-trip).
  - RoPE: the pair-rotation is a partition-half swap.  The q/k feature rows
    are permuted host-side so each rope pair sits inside one 32-partition
    quadrant, which makes the rotation a single DVE stream_shuffle; the sign
    of the second half is folded into the sin rows of the cos/sin constants.
    RMSNorm stats come from PRE-rope values (rotation preserves sum q^2).
  - scores computed TRANSPOSED: scoresT[s, t] = kf.T @ qf so the k-side
    softmax scale folds into the exp's per-partition scale and transposed
    probs feed the PV matmul (lhsT = token-major V) directly.  Softmax
    denominator = accumulated ones-matmuls over the exp tiles.
  - causal mask: diagonal-chunk matmuls are sliced to the unmasked t-range
    and one [128,128] triangle of -1e30 is accumulated on the PE.
  - emission is software-pipelined: wo(s-1) matmuls are interleaved between
    the QK chunks of seq s so the PE has independent work while ACT runs the
    exps; the final seq's wo is emitted per half-seq as soon as the
    normalized attention rows are ready.
"""

import sys

sys.path.insert(0, "/opt/trn_rl_repo")

import numpy as np
import ml_dtypes

import concourse.bass as bass
import concourse.tile as tile
from concourse import bacc, mybir
from concourse.bass_utils import run_bass_kernel_spmd

N_TOK, HID, NH, HD = 4096, 2048, 16, 128
SEQ, NSEQ = 1024, 4
NCORES = 8
EPS = 1e-6
F32, BF16 = mybir.dt.float32, mybir.dt.bfloat16
BF = ml_dtypes.bfloat16
AF = mybir.ActivationFunctionType

# swap the halves of every 32-partition quadrant (DVE stream_shuffle mask)
SHUF_ROT = [(i + 16) % 32 for i in range(32)]
# d-permutation putting rope pair (j, j+64) into one quadrant:
# partition 32q+i   <- x1 dim 16q+i   (i < 16)
# partition 32q+16+i<- x2 dim 16q+i
PERM = np.concatenate(
    [
        np.concatenate([16 * q + np.arange(16), 64 + 16 * q + np.arange(16)])
        for q in range(4)
    ]
).astype(np.int64)

N_WARM = 14  # dummy matmuls ramping the PE p-state before real work lands


def build_nc():
    """One SPMD Bass program; all per-core data arrives via ExternalInputs."""
    nc = bacc.Bacc("TRN2", target_bir_lowering=False, debug=False, num_devices=NCORES)

    xt = nc.dram_tensor("xt", [128, 16, N_TOK], BF16, kind="ExternalInput")
    wqk = nc.dram_tensor("wqk", [128, 4, 16, 128], BF16, kind="ExternalInput")
    wvg = nc.dram_tensor("wvg", [128, 16, 258], BF16, kind="ExternalInput")
    wot = nc.dram_tensor("wot", [128, 2, HID], BF16, kind="ExternalInput")
    cs = nc.dram_tensor("cs", [128, 2, SEQ], BF16, kind="ExternalInput")
    csk = nc.dram_tensor("csk", [128, 2, SEQ], BF16, kind="ExternalInput")
    tri = nc.dram_tensor("tri", [128, 128], BF16, kind="ExternalInput")
    idn = nc.dram_tensor("idn", [128, 128], BF16, kind="ExternalInput")
    gbc = nc.dram_tensor("gbc", [128, 2], F32, kind="ExternalInput")
    out = nc.dram_tensor("out", [N_TOK, HID], BF16, kind="ExternalOutput")

    from contextlib import ExitStack

    with tile.TileContext(nc) as tc:
        with ExitStack() as stack:
            pool = lambda name, bufs, **kw: stack.enter_context(
                tc.tile_pool(name=name, bufs=bufs, **kw)
            )
            consts = pool("consts", 1)
            xtp = pool("xtp", 3)
            qkp = pool("qkp", 2)
            vp = pool("vp", 2)
            finp = pool("finp", 4)
            sqp = pool("sqp", 4)
            rotp = pool("rotp", 2)
            scrp = pool("scrp", 2)
            exp0p = pool("exp0p", 2)
            exp1p = pool("exp1p", 2)
            attnp = pool("attnp", 2)
            obp = pool("obp", 3)
            bcp = pool("bcp", 2)
            rowp = pool("rowp", 3)
            kcp = pool("kcp", 3)
            gsp = pool("gsp", 3)
            gps = pool("gps", 2, space="PSUM")
            bigps = pool("bigps", 3, space="PSUM")
            vecps = pool("vecps", 3, space="PSUM")
            # ---- resident constants
            wqk_t = consts.tile([128, 4, 16, 128], BF16)
            wvg_t = consts.tile([128, 16, 258], BF16)
            wot_t = consts.tile([128, 2, HID], BF16)
            cs_t = consts.tile([128, 2, SEQ], BF16)
            csk_t = consts.tile([128, 2, SEQ], BF16)
            tri_t = consts.tile([128, 128], BF16)
            idn_t = consts.tile([128, 128], BF16)
            gbn_t = consts.tile([128, 2], F32)

            ones_bf = consts.tile([128, 1], BF16)
            nc.vector.memset(ones_bf[:], 1.0)
            ones_f = consts.tile([128, 1], F32)
            nc.vector.memset(ones_f[:], 1.0)
            ones_q = consts.tile([128, 1], BF16)  # 2^-7 exact: stats mm -> mean
            nc.vector.memset(ones_q[:], 1.0 / HD)
            eps_t = consts.tile([128, 1], F32)
            nc.vector.memset(eps_t[:], EPS)
            epsh_t = consts.tile([128, 1], F32)
            nc.vector.memset(epsh_t[:], float(HD * EPS))
            wu_t = consts.tile([128, 256], BF16)
            nc.vector.memset(wu_t[:], 0.001)

            # warmup: ramp the PE p-state while the first DMAs land
            for w in range(N_WARM):
                ps = gps.tile([128, 256], F32, tag="proj", name=f"wu{w}")
                nc.tensor.matmul(
                    ps[:], lhsT=wu_t[:, 0:128], rhs=wu_t[:], start=True, stop=True,
                    skip_group_check=True,
                )

            # ---- DMA staging helpers (emission order tunes queue priority)
            def early_consts():
                nc.scalar.dma_start(out=gbn_t[:], in_=gbc[:])
                for m in range(1, 4):
                    nc.scalar.dma_start(out=wqk_t[:, m], in_=wqk[:, m])
                nc.scalar.dma_start(out=wvg_t[:], in_=wvg[:])

            def late_consts():
                nc.scalar.dma_start(out=cs_t[:], in_=cs[:])
                nc.scalar.dma_start(out=csk_t[:], in_=csk[:])
                nc.scalar.dma_start(out=tri_t[:], in_=tri[:])
                nc.scalar.dma_start(out=idn_t[:], in_=idn[:])
                nc.scalar.dma_start(out=wot_t[:], in_=wot[:])

            xtiles = {}

            def load_xtile(nt):
                if nt in xtiles:
                    return xtiles[nt]
                xtile = xtp.tile([128, 16, 512], BF16, tag="xtile", name=f"xt{nt}")
                bounds = (0, 2, 4, 8, 12, 16) if nt == 0 else (0, 4, 8, 12, 16)
                for a, b in zip(bounds, bounds[1:]):
                    nc.sync.dma_start(
                        out=xtile[:, a:b, :],
                        in_=xt[:, a:b, nt * 512 : (nt + 1) * 512],
                    )
                xtiles[nt] = xtile
                return xtile

            # round-robin copy engines for psum->sbuf moves
            # (GPSIMD/Pool cannot access PSUM on this hw)
            _cp = [0]

            def copy_rr(out_ap, in_ap):
                if _cp[0] % 2 == 0:
                    nc.vector.tensor_copy(out=out_ap, in_=in_ap)
                else:
                    nc.scalar.copy(out=out_ap, in_=in_ap)
                _cp[0] += 1

            def qkv_ntile(nt, qk, vt, gsb):
                """project 512 tokens: q,k feature-major; v+gates token-major."""
                half = nt % 2
                xtile = load_xtile(nt)
                for m in range(4):  # q_h0, q_h1, k_h0, k_h1
                    ps = gps.tile([128, 512], F32, tag="proj")
                    for kc in range(16):
                        nc.tensor.matmul(
                            ps[:],
                            lhsT=wqk_t[:, m, kc, :],
                            rhs=xtile[:, kc, :],
                            start=(kc == 0),
                            stop=(kc == 15),
                        )
                    copy_rr(qk[:, m, half * 512 : (half + 1) * 512], ps[:])
                for ti in range(4):  # v + gate logits, token-major, 128 tok each
                    ps = gps.tile([128, 512], F32, tag="proj")
                    for kc in range(16):
                        nc.tensor.matmul(
                            ps[:, 0:258],
                            lhsT=xtile[:, kc, ti * 128 : (ti + 1) * 128],
                            rhs=wvg_t[:, kc, :],
                            start=(kc == 0),
                            stop=(kc == 15),
                        )
                    copy_rr(vt[:, half * 4 + ti, :], ps[:, 0:256])
                    for h in range(2):  # gate as 1+exp(-(z+b)) = 1/sigmoid:
                        # shares the Exp table with attention; the reciprocal
                        # is folded into the denominator reciprocal later
                        nc.scalar.activation(
                            out=gsb[:, ti * 2 + h : ti * 2 + h + 1],
                            in_=ps[:, 256 + h : 257 + h],
                            func=AF.Exp,
                            bias=gbn_t[:, h : h + 1],
                            scale=-1.0,
                        )
                    nc.vector.tensor_scalar_add(
                        out=gsb[:, ti * 2 : ti * 2 + 2],
                        in0=gsb[:, ti * 2 : ti * 2 + 2],
                        scalar1=ones_f[:],
                    )

            def rope_emit(s, qk, sqs, fins):
                """RoPE + stats squares for all 4 head-tensors of seq s.
                Emitted after qkv(2s+1): sq first (unblocks stats matmuls),
                then per-m rotate+combine; k-side combines go to Pool."""
                for m in range(4):
                    nc.vector.tensor_mul(
                        out=sqs[m][:], in0=qk[:, m, :], in1=qk[:, m, :]
                    )
                for m in (0, 2, 1, 3):  # h0 q,k first: QK(h0,tt0) unblocks early
                    is_q = m < 2
                    cst = cs_t if is_q else csk_t
                    eng = nc.vector if is_q else nc.gpsimd
                    rot = rotp.tile([128, SEQ], BF16, tag="rot")
                    nc.vector.stream_shuffle(rot[:], qk[:, m, :], SHUF_ROT)
                    fin = fins[m]
                    tmp = scrp.tile([128, SEQ], BF16, tag="rtmp")
                    eng.tensor_mul(out=fin[:], in0=qk[:, m, :], in1=cst[:, 0, :])
                    eng.tensor_mul(out=tmp[:], in0=rot[:], in1=cst[:, 1, :])
                    eng.tensor_add(out=fin[:], in0=fin[:], in1=tmp[:])

            def stats_emit(s, sqs, fins, kcols):
                """q: sigma_q broadcast-multiplied into fin (bf16 rows).
                k: kcol[128, 8] per head, consumed by the exp scale."""
                for h in range(2):
                    for half in range(2):
                        js = slice(half * 512, half * 512 + 512)
                        qrow = rowp.tile(
                            [1, 512], F32, tag="qrow", name=f"qr{s}_{h}_{half}"
                        )
                        pss = vecps.tile([1, 512], F32, tag="vec")
                        nc.tensor.matmul(
                            pss[:], lhsT=ones_q[:], rhs=sqs[h][:, js],
                            start=True, stop=True,
                        )
                        nc.scalar.activation(
                            out=qrow[:], in_=pss[:], func=AF.Sqrt,
                            bias=eps_t[0:1, :], scale=1.0,
                        )
                        nc.vector.reciprocal(out=qrow[:], in_=qrow[:])
                        bcq = bcp.tile([128, 512], F32, tag="bcq")
                        nc.gpsimd.partition_broadcast(bcq[:], qrow[:])
                        nc.vector.tensor_mul(
                            out=fins[h][:, js], in0=fins[h][:, js], in1=bcq[:]
                        )
                for h in range(2):
                    kcol = kcols[h]
                    for half in range(2):
                        psc = vecps.tile([128, 4], F32, tag="vec")
                        for b in range(4):
                            sc = half * 4 + b
                            nc.tensor.matmul(
                                psc[:, b : b + 1],
                                lhsT=sqs[2 + h][:, sc * 128 : (sc + 1) * 128],
                                rhs=ones_bf[:],
                                start=True,
                                stop=True,
                                skip_group_check=True,
                            )
                        nc.scalar.activation(
                            out=kcol[:, half * 4 : half * 4 + 4], in_=psc[:],
                            func=AF.Sqrt, bias=epsh_t[:], scale=1.0,
                        )
                        nc.vector.reciprocal(
                            out=kcol[:, half * 4 : half * 4 + 4],
                            in_=kcol[:, half * 4 : half * 4 + 4],
                        )

            def make_wo_units(s, att, eng_cycle):
                """wo(s) as a list of emission thunks: 8 token-blocks x 4
                feature-tiles, two matmuls each, cast-copy to bf16 staging,
                one coarse DMA per token-block."""
                units = []
                obs = {}

                def unit(t8, ot, eng):
                    def emit():
                        if t8 not in obs:
                            obs[t8] = obp.tile(
                                [128, HID], BF16, tag="ob", name=f"ob{s}_{t8}"
                            )
                        ob = obs[t8]
                        ts_ = slice(t8 * 128, (t8 + 1) * 128)
                        os_ = slice(ot * 512, (ot + 1) * 512)
                        ps = gps.tile([128, 512], F32, tag="proj")
                        for h in range(2):
                            nc.tensor.matmul(
                                ps[:],
                                lhsT=att[:, h, ts_],
                                rhs=wot_t[:, h, os_],
                                start=(h == 0),
                                stop=(h == 1),
                            )
                        if eng is nc.scalar:
                            nc.scalar.copy(out=ob[:, os_], in_=ps[:])
                        else:
                            eng.tensor_copy(out=ob[:, os_], in_=ps[:])
                        if ot == 3:
                            nc.sync.dma_start(
                                out=out[s * SEQ + t8 * 128 : s * SEQ + (t8 + 1) * 128, :],
                                in_=ob[:],
                            )

                    return emit

                i = 0
                for t8 in range(8):
                    for ot in range(4):
                        units.append(unit(t8, ot, eng_cycle[i % len(eng_cycle)]))
                        i += 1
                return units

            def qk_chunk(h, tt, sc, fins, kcols, expt):
                """one transposed-scores chunk + exp"""
                r = sc - 4 * tt
                c0 = 128 * r if r > 0 else 0
                sps = bigps.tile([128, 512], F32, tag="big")
                nc.tensor.matmul(
                    sps[:, 0 : 512 - c0],
                    lhsT=fins[2 + h][:, sc * 128 : (sc + 1) * 128],
                    rhs=fins[h][:, tt * 512 + c0 : (tt + 1) * 512],
                    start=True,
                    stop=(r < 0),
                    skip_group_check=True,
                )
                if r >= 0:  # diagonal chunk: add the -1e30 triangle on the PE
                    nc.tensor.matmul(
                        sps[:, 0:128],
                        lhsT=idn_t[:],
                        rhs=tri_t[:],
                        start=False,
                        stop=True,
                        skip_group_check=True,
                    )
                nc.scalar.activation(
                    out=expt[:, sc, c0:512], in_=sps[:, 0 : 512 - c0],
                    func=AF.Exp, scale=kcols[h][:, sc : sc + 1],
                )

            def pv_group(s, h, tt, vt, att, expt, gsbs):
                """PV + denominator + gate transpose + normalize for (h, tt)."""
                nsc = 4 * (tt + 1)
                pv = bigps.tile([128, 512], F32, tag="big", name=f"pv{s}_{h}_{tt}")
                for sc in range(nsc):
                    r = sc - 4 * tt
                    c0 = 128 * r if r > 0 else 0
                    nc.tensor.matmul(
                        pv[:, c0:512],
                        lhsT=vt[:, sc, h * 128 : (h + 1) * 128],
                        rhs=expt[:, sc, c0:512],
                        start=(sc == 0),
                        stop=(sc == nsc - 1),
                        skip_group_check=True,
                    )
                den = vecps.tile([1, 512], F32, tag="vec")
                for sc in range(nsc):
                    r = sc - 4 * tt
                    c0 = 128 * r if r > 0 else 0
                    nc.tensor.matmul(
                        den[:, c0:512],
                        lhsT=ones_bf[:],
                        rhs=expt[:, sc, c0:512],
                        start=(sc == 0),
                        stop=(sc == nsc - 1),
                        skip_group_check=True,
                    )
                sig = vecps.tile([1, 512], F32, tag="vec")
                gsb = gsbs[tt]
                for ti in range(4):  # gates token-major -> free-major rows
                    nc.tensor.matmul(
                        sig[:, ti * 128 : (ti + 1) * 128],
                        lhsT=gsb[:, ti * 2 + h : ti * 2 + h + 1],
                        rhs=idn_t[:],
                        start=True,
                        stop=True,
                        skip_group_check=True,
                    )
                with tc.high_priority():
                    rden = rowp.tile([1, 512], F32, tag="row", name=f"rd{s}_{h}_{tt}")
                    nc.vector.reciprocal(out=rden[:], in_=den[:])
                    sigr = rowp.tile([1, 512], F32, tag="row", name=f"sg{s}_{h}_{tt}")
                    nc.vector.reciprocal(out=sigr[:], in_=sig[:])
                    drec = rowp.tile([1, 512], F32, tag="row", name=f"dr{s}_{h}_{tt}")
                    nc.vector.tensor_mul(out=drec[:], in0=rden[:], in1=sigr[:])
                    bcg = bcp.tile([128, 512], F32, tag="bcg")
                    nc.gpsimd.partition_broadcast(bcg[:], drec[:])
                    nc.vector.tensor_mul(
                        out=att[:, h, tt * 512 : (tt + 1) * 512], in0=pv[:], in1=bcg[:]
                    )

            # ================= main pipeline =================
            att_prev = None
            for s in range(NSEQ):
                qk = qkp.tile([128, 4, SEQ], BF16, tag="qk", name=f"qk{s}")
                vt = vp.tile([128, 8, 256], BF16, tag="v", name=f"v{s}")
                att = attnp.tile([128, 2, SEQ], BF16, tag="attn")
                gsbs = [
                    gsp.tile([128, 8], BF16, tag="gsb", name=f"g{2 * s + t}")
                    for t in range(2)
                ]
                sqs = [sqp.tile([128, SEQ], BF16, tag="sq", name=f"sq{s}_{i}") for i in range(4)]
                fins = [finp.tile([128, SEQ], BF16, tag="fin", name=f"fin{s}_{i}") for i in range(4)]
                kcols = [kcp.tile([128, 8], F32, tag="kcol", name=f"kc{s}_{i}") for i in range(2)]

                if s == 0:
                    nc.sync.dma_start(out=wqk_t[:, 0], in_=wqk[:, 0])
                    load_xtile(0)
                    early_consts()
                qkv_ntile(2 * s, qk, vt, gsbs[0])
                if s == 0:
                    late_consts()
                qkv_ntile(2 * s + 1, qk, vt, gsbs[1])
                if s + 1 < NSEQ:
                    load_xtile(2 * (s + 1))
                    load_xtile(2 * (s + 1) + 1)

                rope_emit(s, qk, sqs, fins)

                # wo(s-1): first token-blocks as a PE block (covers the DVE
                # rope latency), the rest interleaved between QK chunks
                if att_prev is not None:
                    wo_units = make_wo_units(
                        s - 1, att_prev,
                        (nc.scalar, nc.vector, nc.scalar, nc.vector),
                    )
                else:
                    wo_units = []
                for u in wo_units[0:12]:
                    u()
                wo_rest = wo_units[12:]

                stats_emit(s, sqs, fins, kcols)

                expts = {}
                for h in range(2):
                    expts[(h, 0)] = exp0p.tile(
                        [128, 4, 512], BF16, tag="e0", name=f"e0_{s}_{h}"
                    )
                    expts[(h, 1)] = exp1p.tile(
                        [128, 8, 512], BF16, tag="e1", name=f"e1_{s}_{h}"
                    )

                wi = 0
                chunks = (
                    [(0, 0, sc) for sc in range(4)]
                    + [(1, 0, sc) for sc in range(4)]
                    + [(0, 1, sc) for sc in range(8)]
                    + [(1, 1, sc) for sc in range(8)]
                )
                for h, tt, sc in chunks:
                    qk_chunk(h, tt, sc, fins, kcols, expts[(h, tt)])
                    if wi < len(wo_rest):
                        wo_rest[wi]()
                        wi += 1
                for u in wo_rest[wi:]:
                    u()

                for h, tt in ((0, 0), (1, 0), (0, 1), (1, 1)):
                    pv_group(s, h, tt, vt, att, expts[(h, tt)], gsbs)
                    if s == NSEQ - 1 and (h, tt) == (1, 0):
                        last_units = make_wo_units(
                            s, att, (nc.scalar, nc.vector)
                        )
                        for u in last_units[0:16]:
                            u()
                if s == NSEQ - 1:
                    for u in last_units[16:]:
                        u()
                att_prev = att

    if not nc.is_finalized():
        nc.finalize()
    return nc


_NC_CACHE = None


def _get_nc():
    global _NC_CACHE
    if _NC_CACHE is None:
        _NC_CACHE = build_nc()
    return _NC_CACHE


def prep_inputs(x, Wqkv, Wo, gate_w, gate_b, norm_w, cos_cache, sin_cache,
                cu_seqlens, max_seqlen, position_ids):
    x = np.asarray(x, np.float32)
    Wqkv = np.asarray(Wqkv, np.float32)
    Wo = np.asarray(Wo, np.float32)
    gate_w = np.asarray(gate_w, np.float32)
    gate_b = np.asarray(gate_b, np.float32)
    norm_w = np.asarray(norm_w, np.float32)
    cos_cache = np.asarray(cos_cache, np.float32)
    sin_cache = np.asarray(sin_cache, np.float32)
    pid = np.asarray(position_ids).astype(np.int64)
    cu = np.asarray(cu_seqlens).astype(np.int64)
    assert int(max_seqlen) == SEQ and x.shape == (N_TOK, HID)
    assert np.array_equal(cu, np.arange(NSEQ + 1, dtype=np.int64) * SEQ)
    assert np.array_equal(pid, np.tile(np.arange(SEQ, dtype=np.int64), NSEQ))

    xtf = np.ascontiguousarray(x.T).reshape(16, 128, N_TOK).transpose(1, 0, 2)
    xtf = np.ascontiguousarray(xtf).astype(BF)

    # cos/sin rows in the quadrant-permuted d layout; sin negated on x2 rows
    C = cos_cache[pid[:SEQ]].T  # [64, 1024]
    S = sin_cache[pid[:SEQ]].T
    pair = PERM % 64
    sign = np.where(PERM < 64, 1.0, -1.0).astype(np.float32)
    cs0 = C[pair]
    cs1 = S[pair] * sign[:, None]
    csf = np.stack([cs0, cs1], axis=1).astype(BF)
    w2 = (norm_w * norm_w)[PERM].reshape(128, 1).astype(np.float32)
    cskf = (csf.astype(np.float32) * w2[:, None, :]).astype(BF)

    trif = np.where(
        np.arange(128)[:, None] > np.arange(128)[None, :], np.float32(-1e30), 0.0
    ).astype(BF)
    idnf = np.eye(128, dtype=np.float32).astype(BF)

    in_maps = []
    for c in range(NCORES):
        hs = [2 * c, 2 * c + 1]
        rows = []
        for t in range(2):  # q, k row blocks: apply the rope d-permutation
            for h in hs:
                rows.extend((t * HID + h * HD + PERM).tolist())
        for h in hs:  # v rows unpermuted
            rows.extend(range(2 * HID + h * HD, 2 * HID + (h + 1) * HD))
        wsel = np.concatenate([Wqkv[rows], gate_w[hs]], axis=0)  # [770, 2048]
        wall = np.ascontiguousarray(wsel.T).reshape(16, 128, 770).transpose(1, 0, 2)
        wqkf = np.ascontiguousarray(
            wall[:, :, 0:512].reshape(128, 16, 4, 128).transpose(0, 2, 1, 3)
        ).astype(BF)  # [128, 4(m), 16(kc), 128]
        wvgf = np.ascontiguousarray(wall[:, :, 512:770]).astype(BF)
        wo_sl = np.ascontiguousarray(Wo[:, c * 256 : (c + 1) * 256].T)
        wotf = wo_sl.reshape(2, 128, HID).transpose(1, 0, 2)
        wotf = np.ascontiguousarray(wotf).astype(BF)
        gbf = np.broadcast_to(-gate_b[hs][None, :], (128, 2)).astype(np.float32)
        gbf = np.ascontiguousarray(gbf)
        in_maps.append(
            {"xt": xtf, "wqk": wqkf, "wvg": wvgf, "wot": wotf, "cs": csf,
             "csk": cskf, "tri": trif, "idn": idnf, "gbc": gbf}
        )
    return in_maps


def run(inputs, trace=False):
    in_maps = prep_inputs(**inputs)
    nc = _get_nc()
    res = run_bass_kernel_spmd(nc, in_maps, core_ids=list(range(NCORES)), trace=trace)
    total = np.zeros((N_TOK, HID), np.float32)
    for c in range(NCORES):
        total += res.results[c]["out"].astype(np.float32)
    return total, res


def kernel(**inputs) -> np.ndarray:
    out, _ = run(inputs)
    return out
